# revision 3
# baseline (speedup 1.0000x reference)
"""Trainium2 Bass kernel for nn_Minerva2 (pooling / cubic-score attention).

Math:
  Xw = X @ Wx_w.T + Wx_b          [B, Nx, Drep]
  Dw = D @ Wd_w.T + Wd_b          [B, Nd, Drep]
  a  = Xw @ Dw.T                  [B, Nx, Nd]
  act = sign(a)*|a|^3 = a^3
  echo = act @ R                  [B, Nx, 1]
  out = echo * Wr_w + Wr_b

Key identities:
  a^3 * R_d = (a * cbrt(R_d))^3, so cbrt(R) is folded into D's rows on the
  host and the on-chip epilogue is a plain cube + row-sum.
  Biases enter via one K=1 outer-product matmul per accumulation group
  (extra "bias" row in the transposed operands), so every matmul operand is
  DMA-produced float32r (full PE rate at N=512).

Sharding: 8 cores = (batch b, half of Nx). No collectives.
Host passes feature-major (transposed) tensors so every matmul contracts
over the SBUF partition dim with zero on-chip transposes:
  xt  [K+1, NXS]  = X[b, half].T        with row K = 1.0
  dt  [K+1, ND]   = (D[b]*cbrt(R[b])).T with row K = cbrt(R[b])
  wxt [K+1, DREP] = Wx_w.T              with row K = Wx_b
  wdt [K+1, DREP] = Wd_w.T              with row K = Wd_b
"""

import numpy as np

import concourse.bacc as bacc
import concourse.mybir as mybir
import concourse.tile as tile
from concourse.bass_utils import run_bass_kernel_spmd

F32 = mybir.dt.float32
F32R = mybir.dt.float32r
AF = mybir.ActivationFunctionType
ALU = mybir.AluOpType


def build_nc(NXS, ND, DREP, K, wr_w, wr_b):
    """Build the per-core Bass program. All sizes are per-core."""
    KT = K // 128      # k-tiles (contraction over Din)
    RT = DREP // 128   # r-tiles (contraction over Drep in the score matmul)
    DC = ND // 512     # d-chunks of the score matrix
    XC = NXS // 512    # x-chunks
    XT = 4             # x-tiles (128) per x-chunk

    nc = bacc.Bacc("TRN2")
    xt_d = nc.dram_tensor("xt", [K + 1, NXS], F32R, kind="ExternalInput")
    dt_d = nc.dram_tensor("dt", [K + 1, ND], F32R, kind="ExternalInput")
    wxt_d = nc.dram_tensor("wxt", [K + 1, DREP], F32R, kind="ExternalInput")
    wdt_d = nc.dram_tensor("wdt", [K + 1, DREP], F32R, kind="ExternalInput")
    out_d = nc.dram_tensor("out", [NXS, 1], F32, kind="ExternalOutput")

    with tile.TileContext(nc) as tc:
        with (
            tc.tile_pool(name="dwt_pool", bufs=1) as dwt_pool,
            tc.tile_pool(name="psum", bufs=8, space="PSUM") as psum_pool,
            tc.tile_pool(name="misc", bufs=1) as misc_pool,
            tc.tile_pool(name="epi", bufs=2) as epi_pool,
        ):
            # DwT resident: one [128, ND] tile per r-tile
            dwt = [
                dwt_pool.tile([128, ND], F32R, name=f"dwt{r}", tag=f"dwt{r}")
                for r in range(RT)
            ]
            # bias rows (K=1 matmul operands)
            wxt_b = misc_pool.tile([1, DREP], F32R, name="wxt_b")
            nc.sync.dma_start(wxt_b[:], wxt_d[K:K + 1, :])
            wdt_b = misc_pool.tile([1, DREP], F32R, name="wdt_b")
            nc.sync.dma_start(wdt_b[:], wdt_d[K:K + 1, :])

            # ---------------- Phase D: DwT = (Wd D'^T + bd c^T) ----------------
            with (
                tc.tile_pool(name="wdt_sb", bufs=3) as wdt_sb,
                tc.tile_pool(name="dstream", bufs=3) as dstream,
                tc.tile_pool(name="brow", bufs=2) as brow,
            ):
                for c in range(DC):
                    cs = slice(c * 512, (c + 1) * 512)
                    psums = [
                        psum_pool.tile([128, 512], F32, name=f"pd{c}_{r}", tag="ps")
                        for r in range(RT)
                    ]
                    for k in range(KT):
                        wdtk = wdt_sb.tile([128, DREP], F32R, name=f"wdt{c}_{k}",
                                           tag="wdt_stream")
                        nc.sync.dma_start(wdtk[:], wdt_d[k * 128:(k + 1) * 128, :])
                        dtk = dstream.tile([128, 512], F32R, name=f"dt{c}_{k}",
                                           tag="dt_stream")
                        nc.sync.dma_start(dtk[:], dt_d[k * 128:(k + 1) * 128, cs])
                        for r in range(RT):
                            nc.tensor.matmul(
                                psums[r][:],
                                wdtk[:, r * 128:(r + 1) * 128],
                                dtk[:],
                                start=(k == 0), stop=False,
                            )
                    dt_bc = brow.tile([1, 512], F32R, name=f"dt_b{c}", tag="dt_b")
                    nc.sync.dma_start(dt_bc[:], dt_d[K:K + 1, cs])
                    for r in range(RT):
                        # bias outer product bd[r] * c[d], K=1
                        nc.tensor.matmul(
                            psums[r][:],
                            wdt_b[:, r * 128:(r + 1) * 128],
                            dt_bc[:],
                            start=False, stop=True,
                        )
                        nc.vector.tensor_copy(dwt[r][:, cs], psums[r][:])

            # ---------------- Phase X + S per x-chunk ----------------
            with (
                tc.tile_pool(name="wxt_stream", bufs=2) as wxt_stream,
                tc.tile_pool(name="xt_stream", bufs=3) as xt_stream,
                tc.tile_pool(name="xwt_pool", bufs=1) as xwt_pool,
            ):
                for xc in range(XC):
                    xs = slice(xc * 512, (xc + 1) * 512)
                    # --- projection XwT chunk [DREP, 512] ---
                    xwt = [
                        xwt_pool.tile([128, 512], F32R, name=f"xwt{xc}_{r}",
                                      tag=f"xwt{r}")
                        for r in range(RT)
                    ]
                    psums = [
                        psum_pool.tile([128, 512], F32, name=f"px{xc}_{r}", tag="ps")
                        for r in range(RT)
                    ]
                    for k in range(KT):
                        wxtk = wxt_stream.tile([128, DREP], F32R,
                                               name=f"wxt{xc}_{k}", tag="wxt_stream")
                        nc.sync.dma_start(wxtk[:], wxt_d[k * 128:(k + 1) * 128, :])
                        xtk = xt_stream.tile([128, 512], F32R, name=f"xt{xc}_{k}",
                                             tag="xt_stream")
                        nc.sync.dma_start(xtk[:], xt_d[k * 128:(k + 1) * 128, xs])
                        for r in range(RT):
                            nc.tensor.matmul(
                                psums[r][:],
                                wxtk[:, r * 128:(r + 1) * 128],
                                xtk[:],
                                start=(k == 0), stop=False,
                            )
                    xt_bc = xt_stream.tile([1, 512], F32R, name=f"xt_b{xc}",
                                           tag="xt_b")
                    nc.sync.dma_start(xt_bc[:], xt_d[K:K + 1, xs])
                    for r in range(RT):
                        # bias outer product bx[r] * 1, K=1
                        nc.tensor.matmul(
                            psums[r][:],
                            wxt_b[:, r * 128:(r + 1) * 128],
                            xt_bc[:],
                            start=False, stop=True,
                        )
                        nc.vector.tensor_copy(xwt[r][:], psums[r][:])

                    # --- score + cube + reduce per x-tile ---
                    for xi in range(XT):
                        xts = slice(xi * 128, (xi + 1) * 128)
                        gx = xc * 512 + xi * 128
                        spsum = [
                            psum_pool.tile([128, 512], F32, name=f"s{xc}_{xi}_{d}",
                                           tag="ps")
                            for d in range(DC)
                        ]
                        for r in range(RT):
                            for d in range(DC):
                                nc.tensor.matmul(
                                    spsum[d][:],
                                    xwt[r][:, xts],
                                    dwt[r][:, d * 512:(d + 1) * 512],
                                    start=(r == 0), stop=(r == RT - 1),
                                )
                        acc = epi_pool.tile([128, DC], F32, name=f"acc{xc}_{xi}",
                                            tag="acc")
                        for d in range(DC):
                            sq = epi_pool.tile([128, 512], F32,
                                               name=f"sq{xc}_{xi}_{d}", tag="sq")
                            nc.scalar.activation(sq[:], spsum[d][:], AF.Square)
                            t3 = epi_pool.tile([128, 512], F32,
                                               name=f"t3{xc}_{xi}_{d}", tag="t3")
                            nc.vector.scalar_tensor_tensor(
                                out=t3[:], in0=sq[:], scalar=1.0, in1=spsum[d][:],
                                op0=ALU.mult, op1=ALU.mult,
                                accum_out=acc[:, d:d + 1],
                            )
                        echo = epi_pool.tile([128, 1], F32, name=f"echo{xc}_{xi}",
                                             tag="echo")
                        nc.vector.reduce_sum(echo[:], acc[:],
                                             axis=mybir.AxisListType.X)
                        res = epi_pool.tile([128, 1], F32, name=f"res{xc}_{xi}",
                                            tag="res")
                        nc.vector.tensor_scalar(
                            out=res[:], in0=echo[:],
                            scalar1=float(wr_w), scalar2=float(wr_b),
                            op0=ALU.mult, op1=ALU.add,
                        )
                        nc.sync.dma_start(out_d[gx:gx + 128, :], res[:])

    nc.compile()
    return nc


def make_in_maps(X, D, R, Wx_w, Wx_b, Wd_w, Wd_b, n_cores=8):
    B, Nx, Din = X.shape
    Nd = D.shape[1]
    NXS = Nx * B // n_cores
    crt = np.cbrt(R[..., 0].astype(np.float64)).astype(np.float32)  # [B, Nd]
    wxt = np.concatenate([Wx_w.T, Wx_b[None, :]], axis=0)
    wxt = np.ascontiguousarray(wxt)
    wdt = np.concatenate([Wd_w.T, Wd_b[None, :]], axis=0)
    wdt = np.ascontiguousarray(wdt)
    in_maps = []
    halves = n_cores // B
    for core in range(n_cores):
        b, h = divmod(core, halves)
        xs = X[b, h * NXS:(h + 1) * NXS, :].T
        xt = np.concatenate([xs, np.ones((1, NXS), np.float32)], axis=0)
        dp = D[b] * crt[b][:, None]
        dt = np.concatenate([dp.T, crt[b][None, :]], axis=0)
        in_maps.append({
            "xt": np.ascontiguousarray(xt),
            "dt": np.ascontiguousarray(dt),
            "wxt": wxt,
            "wdt": wdt,
        })
    return in_maps


LAST_RESULT = None


def kernel(X, D, R, Wx_w, Wx_b, Wd_w, Wd_b, Wr_w, Wr_b):
    global LAST_RESULT
    B, Nx, Din = X.shape
    Nd = D.shape[1]
    Drep = Wx_w.shape[0]
    n_cores = 8
    NXS = Nx * B // n_cores

    nc = build_nc(NXS, Nd, Drep, Din, float(Wr_w[0, 0]), float(Wr_b[0]))
    in_maps = make_in_maps(X, D, R, Wx_w, Wx_b, Wd_w, Wd_b, n_cores)
    res = run_bass_kernel_spmd(nc, in_maps, core_ids=list(range(n_cores)))
    LAST_RESULT = res

    out = np.empty((B, Nx, 1), dtype=np.float32)
    halves = n_cores // B
    for core in range(n_cores):
        b, h = divmod(core, halves)
        out[b, h * NXS:(h + 1) * NXS, :] = res.results[core]["out"]
    return out
